# revision 1
# baseline (speedup 1.0000x reference)
"""MultiHeadAttention Trainium2 Bass kernel.

Model: B=2, S=2048, D_MODEL=1024, H=16 heads, Dh=64.
  q/k/v = x @ W.T + b ; scores = (q k^T)/8 masked-softmax ; out = w @ v ; y = out @ Wy.T + by

Sharding: (batch x sequence) data parallel over 8 cores. Core c handles
batch b = c // 4 and query rows [q0, q0+512) with q0 = (c % 4) * 512.
K/V projections are computed (replicated) per batch on each core; attention
and the output projection are computed only for the core's query slice, so
the final output is a pure concatenation of per-core slices.

Layout strategy (float32 matmuls, fp32 PSUM accumulation; PRECISE=False
switches the matmul datapath to float32r — 4x faster PE, ~5e-4 absmax error):
  - Host pre-transposes activations/weights so every matmul contraction dim
    lands on SBUF partitions.
  - Scores are computed transposed, scoresT[k, q], so that the attention
    weight matrix comes out with k on partitions, ready to be the moving
    operand of the AV matmul without any on-chip transpose.
  - Mask folded in as an additive {-100, 0} int8 bias before exp (host also
    pre-transposes the mask to [k, q]); softmax max-subtraction is skipped
    (scores are O(6), no overflow risk) and division by the softmax sum is
    applied after the AV matmul via an extra all-ones column of V, which
    yields the row sums in PSUM partition 64.
"""

import numpy as np

import concourse.bass as bass
import concourse.mybir as mybir
import concourse.tile as tile
from concourse import bacc
from concourse.bass_utils import run_bass_kernel_spmd

F32 = mybir.dt.float32
F32R = mybir.dt.float32r
I8 = mybir.dt.int8

# float32r matmuls run 4x faster on the PE but carry ~12-bit operand rounding
# (~5e-4 end-to-end absmax error vs ~2e-6 for float32). PRECISE=True uses
# float32 everywhere.
PRECISE = True
MMD = F32 if PRECISE else F32R

B, S, D, H, DH = 2, 2048, 1024, 16, 64
QS = 512          # query rows per core
P = 128
KO = D // P       # 8 contraction tiles for the projections
MASK_BIAS = -100.0

_CACHE = {}


def r(ap):
    return ap


def build_program():
    nc = bacc.Bacc("TRN2", target_bir_lowering=False, debug=False, num_devices=8)

    # ---- external I/O (per-core shapes) ----
    qT = nc.dram_tensor("qT", [D, QS], MMD, kind="ExternalInput")       # queries[b].T slice
    kT = nc.dram_tensor("kT", [D, S], MMD, kind="ExternalInput")        # keys[b].T
    vT = nc.dram_tensor("vT", [D, S], MMD, kind="ExternalInput")        # values[b].T
    maskb = nc.dram_tensor("maskb", [H, S, QS], I8, kind="ExternalInput")  # (m-1)*100, [h,k,q]
    WqT = nc.dram_tensor("WqT", [D, D], MMD, kind="ExternalInput")      # (Wq/8).T
    WkT = nc.dram_tensor("WkT", [D, D], MMD, kind="ExternalInput")
    WvT = nc.dram_tensor("WvT", [D, D], MMD, kind="ExternalInput")
    WyT = nc.dram_tensor("WyT", [D, D], MMD, kind="ExternalInput")
    bq = nc.dram_tensor("bq", [P, KO], F32, kind="ExternalInput")       # ((bq+bq2)/8) as [p, m]
    bk = nc.dram_tensor("bk", [P, KO], F32, kind="ExternalInput")
    bv = nc.dram_tensor("bv", [1, D], F32, kind="ExternalInput")
    by = nc.dram_tensor("by", [1, D], F32, kind="ExternalInput")
    y = nc.dram_tensor("y", [QS, D], F32, kind="ExternalOutput")

    # V (with bias) bounced through DRAM: [p, h, kt, d]
    vdram = nc.dram_tensor("vdram", [P, H, S // P, DH], MMD)

    qT_r = qT.rearrange("(ko p) q -> p ko q", p=P)
    kT_r = kT.rearrange("(ko p) s -> p ko s", p=P)
    vT_r = vT.rearrange("(ko p) s -> p ko s", p=P)
    maskb_r = maskb.rearrange("h (kt p) q -> h p kt q", p=P)
    WqT_r = WqT.rearrange("(ko p) m -> p ko m", p=P)
    WkT_r = WkT.rearrange("(ko p) m -> p ko m", p=P)
    WvT_r = WvT.rearrange("(ko p) m -> p ko m", p=P)
    WyT_r = WyT.rearrange("(ko p) m -> p ko m", p=P)

    def bcast_dram(ap, parts):
        # partition-broadcast AP: read the same DRAM row into `parts` partitions
        return bass.AP(tensor=ap.tensor, offset=ap.offset, ap=[[0, parts]] + list(ap.ap[1:]))

    with tile.TileContext(nc) as tc:
        with (
            tc.tile_pool(name="persist", bufs=1) as persist,
            tc.tile_pool(name="w", bufs=1) as wpool,
            tc.tile_pool(name="col", bufs=2) as colpool,
            tc.tile_pool(name="vch", bufs=2) as vchpool,
            tc.tile_pool(name="small", bufs=3) as small,
            tc.tile_pool(name="tiny", bufs=2) as tiny,
            tc.tile_pool(name="maskp", bufs=4) as maskp,
            tc.tile_pool(name="vhp", bufs=2) as vhp,
            tc.tile_pool(name="outp", bufs=2) as outp,
            tc.tile_pool(name="dscr", bufs=2, space="DRAM") as dscr,
            tc.tile_pool(name="psA", bufs=3, space="PSUM") as psA,
            tc.tile_pool(name="psS", bufs=3, space="PSUM") as psS,
            tc.tile_pool(name="psT", bufs=2, space="PSUM") as psT,
        ):
            # ---- persistent SBUF ----
            KT_sb = persist.tile([P, KO, S], MMD)        # 8 MB [dout_p, dout_o, k]
            QT_sb = persist.tile([P, KO, QS], MMD)       # 2 MB
            attnT = persist.tile([P, KO, QS], MMD)       # 2 MB [i_p, i_o, q]
            bq_sb = persist.tile([P, KO], F32)
            bk_sb = persist.tile([P, KO], F32)
            bv_bc = persist.tile([P, D], F32)
            by_bc = persist.tile([P, D], F32)
            nc.sync.dma_start(out=bq_sb, in_=bq[:])
            nc.sync.dma_start(out=bk_sb, in_=bk[:])
            nc.sync.dma_start(out=bv_bc, in_=bcast_dram(bv[:], P))
            nc.sync.dma_start(out=by_bc, in_=bcast_dram(by[:], P))

            # ---- phase K: KT_sb[dout, k] = Wk @ keys[b].T + bk ----
            wk = wpool.tile([P, KO, D], MMD, tag="w")
            nc.sync.dma_start(out=wk, in_=WkT_r[:])
            for nch in range(8):
                kcol = colpool.tile([P, KO, 256], MMD, tag="col")
                nc.sync.dma_start(out=kcol, in_=kT_r[:, :, nch * 256:(nch + 1) * 256])
                for m in range(KO):
                    ps = psA.tile([P, 256], F32, tag="proj")
                    for ko in range(KO):
                        nc.tensor.matmul(
                            ps[:], r(wk[:, ko, m * P:(m + 1) * P]), r(kcol[:, ko, :]),
                            start=(ko == 0), stop=(ko == KO - 1))
                    nc.scalar.activation(
                        out=KT_sb[:, m, nch * 256:(nch + 1) * 256], in_=ps[:],
                        func=mybir.ActivationFunctionType.Identity,
                        bias=bk_sb[:, m:m + 1], scale=1.0)

            # ---- phase V: vdram[p, h, kt, d] = values[b] @ Wv.T + bv ----
            wv = wpool.tile([P, KO, D], MMD, tag="w")
            nc.sync.dma_start(out=wv, in_=WvT_r[:])
            for st in range(S // P):
                vch = vchpool.tile([P, KO, P], MMD, tag="vch")
                nc.sync.dma_start(out=vch, in_=vT_r[:, :, st * P:(st + 1) * P])
                for half in range(2):
                    ps = psA.tile([P, 512], F32, tag="proj")
                    for ko in range(KO):
                        nc.tensor.matmul(
                            ps[:], r(vch[:, ko, :]), r(wv[:, ko, half * 512:(half + 1) * 512]),
                            start=(ko == 0), stop=(ko == KO - 1))
                    vout = outp.tile([P, 512], MMD, tag="vout")
                    nc.vector.tensor_tensor(
                        vout[:], ps[:], bv_bc[:, half * 512:(half + 1) * 512],
                        mybir.AluOpType.add)
                    nc.sync.dma_start(
                        out=vdram[:, half * 8:(half + 1) * 8, st, :],
                        in_=vout.rearrange("p (h d) -> p h d", d=DH))

            # ---- phase Q: QT_sb[dout, q] = (Wq/8) @ queries[b].T + bq/8 ----
            wq = wpool.tile([P, KO, D], MMD, tag="w")
            nc.sync.dma_start(out=wq, in_=WqT_r[:])
            for nch in range(2):
                qcol = colpool.tile([P, KO, 256], MMD, tag="col")
                nc.sync.dma_start(out=qcol, in_=qT_r[:, :, nch * 256:(nch + 1) * 256])
                for m in range(KO):
                    ps = psA.tile([P, 256], F32, tag="proj")
                    for ko in range(KO):
                        nc.tensor.matmul(
                            ps[:], r(wq[:, ko, m * P:(m + 1) * P]), r(qcol[:, ko, :]),
                            start=(ko == 0), stop=(ko == KO - 1))
                    nc.scalar.activation(
                        out=QT_sb[:, m, nch * 256:(nch + 1) * 256], in_=ps[:],
                        func=mybir.ActivationFunctionType.Identity,
                        bias=bq_sb[:, m:m + 1], scale=1.0)

            # ---- phase A: attention per head ----
            for h in range(H):
                hm, hp = h // 2, h % 2
                mbs = []
                for mh in range(2):
                    mb = maskp.tile([P, S // P // 2, QS], I8, tag="mask")
                    nc.sync.dma_start(out=mb, in_=maskb_r[h, :, mh * 8:(mh + 1) * 8, :])
                    mbs.append(mb)
                vh = vhp.tile([P, S // P, DH + 1], MMD, tag="vh")
                nc.sync.dma_start(out=vh[:, :, 0:DH], in_=vdram[:, h, :, :])
                # ones column (f32r): memset doesn't support f32r, so x*0+1
                nc.vector.tensor_scalar(
                    vh[:, :, DH:DH + 1], vh[:, :, 0:1], 0.0, 1.0,
                    mybir.AluOpType.mult, mybir.AluOpType.add)

                patt = psT.tile([P, 512], F32, tag="att")
                qh = QT_sb[hp * DH:(hp + 1) * DH, hm, :]
                for kt in range(S // P):
                    pscr = psS.tile([P, 512], F32, tag="scores")
                    nc.tensor.matmul(
                        pscr[:], r(KT_sb[hp * DH:(hp + 1) * DH, hm, kt * P:(kt + 1) * P]),
                        r(qh), start=True, stop=True)
                    eT = small.tile([P, 512], MMD, tag="eT")
                    nc.vector.tensor_tensor(eT[:], pscr[:], mbs[kt // 8][:, kt % 8, :],
                                            mybir.AluOpType.add)
                    nc.scalar.activation(out=eT[:], in_=eT[:],
                                         func=mybir.ActivationFunctionType.Exp)
                    nc.tensor.matmul(
                        patt[:DH + 1, :], r(vh[:, kt, :]), r(eT[:]),
                        start=(kt == 0), stop=(kt == S // P - 1))

                rec = tiny.tile([1, 512], F32, tag="rec")
                nc.vector.reciprocal(out=rec[:], in_=patt[DH:DH + 1, :])
                recd = dscr.tile([1, 512], F32, tag="recd")
                nc.sync.dma_start(out=recd[:], in_=rec[:])
                rb = tiny.tile([DH, 512], F32, tag="rb")
                nc.sync.dma_start(out=rb, in_=bass.AP(
                    tensor=recd.tensor, offset=recd.offset,
                    ap=[[0, DH]] + list(recd.ap[1:])))
                nc.vector.tensor_tensor(
                    attnT[hp * DH:(hp + 1) * DH, hm, :], patt[0:DH, :], rb[:],
                    mybir.AluOpType.mult)

            # ---- phase Y: y = merged @ Wy.T + by ----
            for nch in range(4):
                ycol = colpool.tile([P, KO, 256], MMD, tag="col")
                nc.sync.dma_start(out=ycol, in_=WyT_r[:, :, nch * 256:(nch + 1) * 256])
                for m in range(4):
                    ps = psA.tile([P, 256], F32, tag="proj")
                    for ko in range(KO):
                        nc.tensor.matmul(
                            ps[:], r(attnT[:, ko, m * P:(m + 1) * P]), r(ycol[:, ko, :]),
                            start=(ko == 0), stop=(ko == KO - 1))
                    ysb = outp.tile([P, 256], F32, tag="vout")
                    nc.vector.tensor_tensor(
                        ysb[:], ps[:], by_bc[:, nch * 256:(nch + 1) * 256],
                        mybir.AluOpType.add)
                    nc.sync.dma_start(
                        out=y[m * P:(m + 1) * P, nch * 256:(nch + 1) * 256], in_=ysb[:])

    nc.compile()
    return nc


def prep_inputs(queries, keys, values, mask, Wq, bq, Wk, bk, Wv, bv, Wy, by,
                bq2, bk2, bv2, by2):
    f = np.float32
    WqT = np.ascontiguousarray((Wq.astype(f) / 8.0).T)
    WkT = np.ascontiguousarray(Wk.astype(f).T)
    WvT = np.ascontiguousarray(Wv.astype(f).T)
    WyT = np.ascontiguousarray(Wy.astype(f).T)
    bq_t = np.ascontiguousarray(((bq + bq2).astype(f) / 8.0).reshape(KO, P).T)
    bk_t = np.ascontiguousarray((bk + bk2).astype(f).reshape(KO, P).T)
    bv_t = np.ascontiguousarray((bv + bv2).astype(f)[None, :])
    by_t = np.ascontiguousarray((by + by2).astype(f)[None, :])

    qT = [np.ascontiguousarray(queries[b].astype(f).T) for b in range(B)]
    kT = [np.ascontiguousarray(keys[b].astype(f).T) for b in range(B)]
    vT = [np.ascontiguousarray(values[b].astype(f).T) for b in range(B)]
    # mask bias: (m-1)*100 in int8, transposed to [h, k, q] per core slice
    mb = ((mask.astype(np.int8) - 1) * 100)  # [B, H, Q, K]

    in_maps = []
    for c in range(8):
        b, qi = c // 4, c % 4
        q0 = qi * QS
        in_maps.append({
            "qT": np.ascontiguousarray(qT[b][:, q0:q0 + QS]),
            "kT": kT[b],
            "vT": vT[b],
            "maskb": np.ascontiguousarray(mb[b, :, q0:q0 + QS, :].transpose(0, 2, 1)),
            "WqT": WqT, "WkT": WkT, "WvT": WvT, "WyT": WyT,
            "bq": bq_t, "bk": bk_t, "bv": bv_t, "by": by_t,
        })
    return in_maps


def kernel(**inputs):
    if "nc" not in _CACHE:
        _CACHE["nc"] = build_program()
    nc = _CACHE["nc"]
    in_maps = prep_inputs(**inputs)
    res = run_bass_kernel_spmd(nc, in_maps, core_ids=list(range(8)))
    out = np.empty((B, S, D), dtype=np.float32)
    for c in range(8):
        b, qi = c // 4, c % 4
        out[b, qi * QS:(qi + 1) * QS, :] = res.results[c]["y"]
    return out



# revision 2
# speedup vs baseline: 1.7194x; 1.7194x over previous
"""MultiHeadAttention Trainium2 Bass kernel.

Model: B=2, S=2048, D_MODEL=1024, H=16 heads, Dh=64.
  q/k/v = x @ W.T + b ; scores = (q k^T)/8 masked-softmax ; out = w @ v ; y = out @ Wy.T + by

Sharding: (batch x sequence) data parallel over 8 cores. Core c handles
batch b = c // 4 and query rows [q0, q0+512) with q0 = (c % 4) * 512.
K/V projections are computed (replicated) per batch on each core; attention
and the output projection are computed only for the core's query slice, so
the final output is a pure concatenation of per-core slices.

Layout strategy (float32 matmuls, fp32 PSUM accumulation; PRECISE=False
switches the matmul datapath to float32r — 4x faster PE, ~5e-4 absmax error):
  - Host pre-transposes activations/weights so every matmul contraction dim
    lands on SBUF partitions.
  - Scores are computed transposed, scoresT[k, q], so that the attention
    weight matrix comes out with k on partitions, ready to be the moving
    operand of the AV matmul without any on-chip transpose.
  - Mask folded in as an additive {-100, 0} int8 bias before exp (host also
    pre-transposes the mask to [k, q]); softmax max-subtraction is skipped
    (scores are O(6), no overflow risk) and division by the softmax sum is
    applied after the AV matmul via an extra all-ones column of V, which
    yields the row sums in PSUM partition 64.
"""

import numpy as np

import concourse.bass as bass
import concourse.mybir as mybir
import concourse.tile as tile
from concourse import bacc
from concourse.bass_utils import run_bass_kernel_spmd

F32 = mybir.dt.float32
F32R = mybir.dt.float32r
I8 = mybir.dt.int8

# float32r matmuls run 4x faster on the PE but carry ~12-bit operand rounding
# (~5e-4 end-to-end absmax error vs ~2e-6 for float32). PRECISE=True uses
# float32 everywhere.
PRECISE = False
MMD = F32 if PRECISE else F32R

B, S, D, H, DH = 2, 2048, 1024, 16, 64
QS = 512          # query rows per core
P = 128
KO = D // P       # 8 contraction tiles for the projections
MASK_BIAS = -100.0

_CACHE = {}


def r(ap):
    return ap


def build_program():
    nc = bacc.Bacc("TRN2", target_bir_lowering=False, debug=False, num_devices=8)

    # ---- external I/O (per-core shapes) ----
    qT = nc.dram_tensor("qT", [D, QS], MMD, kind="ExternalInput")       # queries[b].T slice
    kT = nc.dram_tensor("kT", [D, S], MMD, kind="ExternalInput")        # keys[b].T
    vT = nc.dram_tensor("vT", [D, S], MMD, kind="ExternalInput")        # values[b].T
    maskb = nc.dram_tensor("maskb", [H, S, QS], I8, kind="ExternalInput")  # (m-1)*100, [h,k,q]
    WqT = nc.dram_tensor("WqT", [D, D], MMD, kind="ExternalInput")      # (Wq/8).T
    WkT = nc.dram_tensor("WkT", [D, D], MMD, kind="ExternalInput")
    WvT = nc.dram_tensor("WvT", [D, D], MMD, kind="ExternalInput")
    WyT = nc.dram_tensor("WyT", [D, D], MMD, kind="ExternalInput")
    bq = nc.dram_tensor("bq", [P, KO], F32, kind="ExternalInput")       # ((bq+bq2)/8) as [p, m]
    bk = nc.dram_tensor("bk", [P, KO], F32, kind="ExternalInput")
    bv = nc.dram_tensor("bv", [1, D], F32, kind="ExternalInput")
    by = nc.dram_tensor("by", [1, D], F32, kind="ExternalInput")
    y = nc.dram_tensor("y", [QS, D], F32, kind="ExternalOutput")

    # V (with bias) bounced through DRAM: [p, h, kt, d]
    vdram = nc.dram_tensor("vdram", [P, H, S // P, DH], MMD)

    qT_r = qT.rearrange("(ko p) q -> p ko q", p=P)
    kT_r = kT.rearrange("(ko p) s -> p ko s", p=P)
    vT_r = vT.rearrange("(ko p) s -> p ko s", p=P)
    maskb_r = maskb.rearrange("h (kt p) q -> h p kt q", p=P)
    WqT_r = WqT.rearrange("(ko p) m -> p ko m", p=P)
    WkT_r = WkT.rearrange("(ko p) m -> p ko m", p=P)
    WvT_r = WvT.rearrange("(ko p) m -> p ko m", p=P)
    WyT_r = WyT.rearrange("(ko p) m -> p ko m", p=P)

    def bcast_dram(ap, parts):
        # partition-broadcast AP: read the same DRAM row into `parts` partitions
        return bass.AP(tensor=ap.tensor, offset=ap.offset, ap=[[0, parts]] + list(ap.ap[1:]))

    with tile.TileContext(nc) as tc:
        with (
            tc.tile_pool(name="persist", bufs=1) as persist,
            tc.tile_pool(name="w", bufs=1) as wpool,
            tc.tile_pool(name="col", bufs=2) as colpool,
            tc.tile_pool(name="vch", bufs=2) as vchpool,
            tc.tile_pool(name="small", bufs=3) as small,
            tc.tile_pool(name="tiny", bufs=2) as tiny,
            tc.tile_pool(name="maskp", bufs=4) as maskp,
            tc.tile_pool(name="vhp", bufs=2) as vhp,
            tc.tile_pool(name="outp", bufs=2) as outp,
            tc.tile_pool(name="dscr", bufs=2, space="DRAM") as dscr,
            tc.tile_pool(name="psA", bufs=3, space="PSUM") as psA,
            tc.tile_pool(name="psS", bufs=3, space="PSUM") as psS,
            tc.tile_pool(name="psT", bufs=2, space="PSUM") as psT,
        ):
            # ---- persistent SBUF ----
            KT_sb = persist.tile([P, KO, S], MMD)        # 8 MB [dout_p, dout_o, k]
            QT_sb = persist.tile([P, KO, QS], MMD)       # 2 MB
            attnT = persist.tile([P, KO, QS], MMD)       # 2 MB [i_p, i_o, q]
            bq_sb = persist.tile([P, KO], F32)
            bk_sb = persist.tile([P, KO], F32)
            bv_bc = persist.tile([P, D], F32)
            by_bc = persist.tile([P, D], F32)
            nc.sync.dma_start(out=bq_sb, in_=bq[:])
            nc.sync.dma_start(out=bk_sb, in_=bk[:])
            nc.sync.dma_start(out=bv_bc, in_=bcast_dram(bv[:], P))
            nc.sync.dma_start(out=by_bc, in_=bcast_dram(by[:], P))

            # ---- phase K: KT_sb[dout, k] = Wk @ keys[b].T + bk ----
            wk = wpool.tile([P, KO, D], MMD, tag="w")
            nc.sync.dma_start(out=wk, in_=WkT_r[:])
            for nch in range(8):
                kcol = colpool.tile([P, KO, 256], MMD, tag="col")
                nc.sync.dma_start(out=kcol, in_=kT_r[:, :, nch * 256:(nch + 1) * 256])
                for m in range(KO):
                    ps = psA.tile([P, 256], F32, tag="proj")
                    for ko in range(KO):
                        nc.tensor.matmul(
                            ps[:], r(wk[:, ko, m * P:(m + 1) * P]), r(kcol[:, ko, :]),
                            start=(ko == 0), stop=(ko == KO - 1))
                    nc.scalar.activation(
                        out=KT_sb[:, m, nch * 256:(nch + 1) * 256], in_=ps[:],
                        func=mybir.ActivationFunctionType.Identity,
                        bias=bk_sb[:, m:m + 1], scale=1.0)

            # ---- phase V: vdram[p, h, kt, d] = values[b] @ Wv.T + bv ----
            wv = wpool.tile([P, KO, D], MMD, tag="w")
            nc.sync.dma_start(out=wv, in_=WvT_r[:])
            for st in range(S // P):
                vch = vchpool.tile([P, KO, P], MMD, tag="vch")
                nc.sync.dma_start(out=vch, in_=vT_r[:, :, st * P:(st + 1) * P])
                for half in range(2):
                    ps = psA.tile([P, 512], F32, tag="proj")
                    for ko in range(KO):
                        nc.tensor.matmul(
                            ps[:], r(vch[:, ko, :]), r(wv[:, ko, half * 512:(half + 1) * 512]),
                            start=(ko == 0), stop=(ko == KO - 1))
                    vout = outp.tile([P, 512], MMD, tag="vout")
                    nc.vector.tensor_tensor(
                        vout[:], ps[:], bv_bc[:, half * 512:(half + 1) * 512],
                        mybir.AluOpType.add)
                    nc.sync.dma_start(
                        out=vdram[:, half * 8:(half + 1) * 8, st, :],
                        in_=vout.rearrange("p (h d) -> p h d", d=DH))

            # ---- phase Q: QT_sb[dout, q] = (Wq/8) @ queries[b].T + bq/8 ----
            wq = wpool.tile([P, KO, D], MMD, tag="w")
            nc.sync.dma_start(out=wq, in_=WqT_r[:])
            for nch in range(2):
                qcol = colpool.tile([P, KO, 256], MMD, tag="col")
                nc.sync.dma_start(out=qcol, in_=qT_r[:, :, nch * 256:(nch + 1) * 256])
                for m in range(KO):
                    ps = psA.tile([P, 256], F32, tag="proj")
                    for ko in range(KO):
                        nc.tensor.matmul(
                            ps[:], r(wq[:, ko, m * P:(m + 1) * P]), r(qcol[:, ko, :]),
                            start=(ko == 0), stop=(ko == KO - 1))
                    nc.scalar.activation(
                        out=QT_sb[:, m, nch * 256:(nch + 1) * 256], in_=ps[:],
                        func=mybir.ActivationFunctionType.Identity,
                        bias=bq_sb[:, m:m + 1], scale=1.0)

            # ---- phase A: attention per head ----
            for h in range(H):
                hm, hp = h // 2, h % 2
                mbs = []
                for mh in range(2):
                    mb = maskp.tile([P, S // P // 2, QS], I8, tag="mask")
                    nc.sync.dma_start(out=mb, in_=maskb_r[h, :, mh * 8:(mh + 1) * 8, :])
                    mbs.append(mb)
                vh = vhp.tile([P, S // P, DH + 1], MMD, tag="vh")
                nc.sync.dma_start(out=vh[:, :, 0:DH], in_=vdram[:, h, :, :])
                # ones column (f32r): memset doesn't support f32r, so x*0+1
                nc.vector.tensor_scalar(
                    vh[:, :, DH:DH + 1], vh[:, :, 0:1], 0.0, 1.0,
                    mybir.AluOpType.mult, mybir.AluOpType.add)

                patt = psT.tile([P, 512], F32, tag="att")
                qh = QT_sb[hp * DH:(hp + 1) * DH, hm, :]
                for kt in range(S // P):
                    pscr = psS.tile([P, 512], F32, tag="scores")
                    nc.tensor.matmul(
                        pscr[:], r(KT_sb[hp * DH:(hp + 1) * DH, hm, kt * P:(kt + 1) * P]),
                        r(qh), start=True, stop=True)
                    eT = small.tile([P, 512], MMD, tag="eT")
                    nc.vector.tensor_tensor(eT[:], pscr[:], mbs[kt // 8][:, kt % 8, :],
                                            mybir.AluOpType.add)
                    nc.scalar.activation(out=eT[:], in_=eT[:],
                                         func=mybir.ActivationFunctionType.Exp)
                    nc.tensor.matmul(
                        patt[:DH + 1, :], r(vh[:, kt, :]), r(eT[:]),
                        start=(kt == 0), stop=(kt == S // P - 1))

                rec = tiny.tile([1, 512], F32, tag="rec")
                nc.vector.reciprocal(out=rec[:], in_=patt[DH:DH + 1, :])
                recd = dscr.tile([1, 512], F32, tag="recd")
                nc.sync.dma_start(out=recd[:], in_=rec[:])
                rb = tiny.tile([DH, 512], F32, tag="rb")
                nc.sync.dma_start(out=rb, in_=bass.AP(
                    tensor=recd.tensor, offset=recd.offset,
                    ap=[[0, DH]] + list(recd.ap[1:])))
                nc.vector.tensor_tensor(
                    attnT[hp * DH:(hp + 1) * DH, hm, :], patt[0:DH, :], rb[:],
                    mybir.AluOpType.mult)

            # ---- phase Y: y = merged @ Wy.T + by ----
            for nch in range(4):
                ycol = colpool.tile([P, KO, 256], MMD, tag="col")
                nc.sync.dma_start(out=ycol, in_=WyT_r[:, :, nch * 256:(nch + 1) * 256])
                for m in range(4):
                    ps = psA.tile([P, 256], F32, tag="proj")
                    for ko in range(KO):
                        nc.tensor.matmul(
                            ps[:], r(attnT[:, ko, m * P:(m + 1) * P]), r(ycol[:, ko, :]),
                            start=(ko == 0), stop=(ko == KO - 1))
                    ysb = outp.tile([P, 256], F32, tag="vout")
                    nc.vector.tensor_tensor(
                        ysb[:], ps[:], by_bc[:, nch * 256:(nch + 1) * 256],
                        mybir.AluOpType.add)
                    nc.sync.dma_start(
                        out=y[m * P:(m + 1) * P, nch * 256:(nch + 1) * 256], in_=ysb[:])

    nc.compile()
    return nc


def prep_inputs(queries, keys, values, mask, Wq, bq, Wk, bk, Wv, bv, Wy, by,
                bq2, bk2, bv2, by2):
    f = np.float32
    WqT = np.ascontiguousarray((Wq.astype(f) / 8.0).T)
    WkT = np.ascontiguousarray(Wk.astype(f).T)
    WvT = np.ascontiguousarray(Wv.astype(f).T)
    WyT = np.ascontiguousarray(Wy.astype(f).T)
    bq_t = np.ascontiguousarray(((bq + bq2).astype(f) / 8.0).reshape(KO, P).T)
    bk_t = np.ascontiguousarray((bk + bk2).astype(f).reshape(KO, P).T)
    bv_t = np.ascontiguousarray((bv + bv2).astype(f)[None, :])
    by_t = np.ascontiguousarray((by + by2).astype(f)[None, :])

    qT = [np.ascontiguousarray(queries[b].astype(f).T) for b in range(B)]
    kT = [np.ascontiguousarray(keys[b].astype(f).T) for b in range(B)]
    vT = [np.ascontiguousarray(values[b].astype(f).T) for b in range(B)]
    # mask bias: (m-1)*100 in int8, transposed to [h, k, q] per core slice
    mb = ((mask.astype(np.int8) - 1) * 100)  # [B, H, Q, K]

    in_maps = []
    for c in range(8):
        b, qi = c // 4, c % 4
        q0 = qi * QS
        in_maps.append({
            "qT": np.ascontiguousarray(qT[b][:, q0:q0 + QS]),
            "kT": kT[b],
            "vT": vT[b],
            "maskb": np.ascontiguousarray(mb[b, :, q0:q0 + QS, :].transpose(0, 2, 1)),
            "WqT": WqT, "WkT": WkT, "WvT": WvT, "WyT": WyT,
            "bq": bq_t, "bk": bk_t, "bv": bv_t, "by": by_t,
        })
    return in_maps


def kernel(**inputs):
    if "nc" not in _CACHE:
        _CACHE["nc"] = build_program()
    nc = _CACHE["nc"]
    in_maps = prep_inputs(**inputs)
    res = run_bass_kernel_spmd(nc, in_maps, core_ids=list(range(8)))
    out = np.empty((B, S, D), dtype=np.float32)
    for c in range(8):
        b, qi = c // 4, c % 4
        out[b, qi * QS:(qi + 1) * QS, :] = res.results[c]["y"]
    return out

